# revision 57
# baseline (speedup 1.0000x reference)
"""Trainium2 Bass kernel for nn_MultiHeadAttention_73589969649754
(gnn_message_passing / graph cross-attention).

Strategy (v2):
  - Edges sorted by destination node; core c owns dst nodes
    [c*2500, (c+1)*2500) on each side (aligned with the projection
    shards; Poisson degree keeps the edge imbalance ~1%).
  - Phase A per core: 4 projection GEMMs over the 2500-node shard into
    K|V tables (bf16, interleaved rows), AllGathered so each core holds
    the full [20480, 1024] K|V tables of both sides.
  - Edge phase per window (<=128 dst nodes, <=896 edge slots, 7 blocks):
      * src K gathered channel-major via dma_gather(transpose=True)
        -> [128ch, 4cc, 896e]; src V gathered row-major [e, ch]; the
        window's 128 dst-K rows gathered channel-major (128 idx).
      * scores on the PE: S^T[e,n] = sum_cc KsrcT_cc^T x KdstT_cc per
        block; the per-edge score is extracted by one fused DVE
        scalar_tensor_tensor (mask by the bare one-hot + row-sum into
        s[e]); eh = exp(s/temp) - one ACT op per window (no
        max-subtraction; logits are ~N(0,1)).
      * weighted one-hot ohs = (iota==drel)*eh (DVE tensor_scalar, 4x
        mode); segment-sum via PE one-hot matmul in [ch, node]
        orientation; z via ones x ohs matmul.
      * 1/z broadcast via rank-1 matmul; divide fused into the
        PSUM->SBUF copy; Wo GEMM consumes [ch, node] directly; bias via
        rank-1 matmul; leaky relu as 0.01*h + relu(0.99*h) (Relu/Copy
        share the Exp activation table; Lrelu would force table
        reloads and is not supported by the device runtime here).
  - Outputs accumulate in an 8-window SBUF ring (bf16), flushed every
    2 windows so the store DMA overlaps the tail of the pipeline.
"""

import math

import numpy as np

N = 20000
E = 160000
C = 512
NCORES = 8
TEMP = float(np.sqrt(C))
NEG = 0.01
NPC = N // NCORES            # 2500 nodes per shard
NTILES = math.ceil(NPC / 128)
NPAD = NTILES * 128          # 2560 padded shard rows
BLK = 128                    # edges per block
BPW = 7                      # blocks per window
WCAP = BPW * BLK             # 896 edge slots per window
SCOL = WCAP // 16            # 56 idx columns per window
DCOL = BLK // 16             # 8 idx columns for the dst gather
DUMMY_REL = 999.0
SKIP_AG = False
SKIP_C = False
DBG = set()


def _table_row(n):
    n = np.minimum(np.asarray(n, np.int64), N - 1)
    return (n // NPC) * NPAD + (n % NPC)


def _prep_side(seg_dst, seg_src):
    seg_dst = np.asarray(seg_dst, np.int64)
    seg_src = np.asarray(seg_src, np.int64)
    perm = np.argsort(seg_dst, kind="stable")
    sd = seg_dst[perm]
    ss = seg_src[perm]
    deg = np.bincount(sd, minlength=N)
    assert deg.max() <= WCAP, "node degree exceeds window capacity"

    cores = []
    max_w = 0
    for c in range(NCORES):
        n0, n1 = c * NPC, (c + 1) * NPC
        e0 = int(np.searchsorted(sd, n0, "left"))
        wins = []
        n, e = n0, e0
        while n < n1:
            wn = we = 0
            while n + wn < n1 and wn < BLK and we + deg[n + wn] <= WCAP:
                we += deg[n + wn]
                wn += 1
            assert wn > 0
            wins.append((n, wn, e, we))
            n += wn
            e += we
        assert e == int(np.searchsorted(sd, n1, "left"))
        cores.append((n0, wins, sd, ss))
        max_w = max(max_w, len(wins))
    return cores, max_w


def _wrap_idx16(idx_flat):
    """[n] -> [128, n//16] int16, i at [i%16, i//16], replicated x8."""
    n = idx_flat.shape[0]
    a = idx_flat.reshape(n // 16, 16).T.astype(np.int16)
    return np.ascontiguousarray(np.tile(a, (8, 1)))


def _build_core_arrays(cores, W):
    out = []
    for (n0, wins, sd, ss) in cores:
        sidxK = np.zeros((128, W * SCOL), np.int16)
        didx = np.zeros((128, W * DCOL), np.int16)
        drel = np.full((128, W * BPW), DUMMY_REL, np.float32)
        colnode = np.full(W * BLK, -1, np.int64)
        for w, (fn, wn, es, we) in enumerate(wins):
            rows = _table_row(ss[es:es + we])
            padK = np.zeros(WCAP, np.int64)
            padK[:we] = rows
            sidxK[:, w * SCOL:(w + 1) * SCOL] = _wrap_idx16(padK)
            didx[:, w * DCOL:(w + 1) * DCOL] = _wrap_idx16(
                _table_row(np.arange(fn, fn + BLK)))
            rel = np.full(WCAP, DUMMY_REL, np.float32)
            rel[:we] = (sd[es:es + we] - fn).astype(np.float32)
            drel[:, w * BPW:(w + 1) * BPW] = rel.reshape(BPW, BLK).T
            colnode[w * BLK: w * BLK + wn] = np.arange(fn, fn + wn)
        out.append(dict(sidxK=sidxK, didx=didx, drel=drel,
                        colnode=colnode))
    return out


def _build_program(W):
    import concourse.bacc as bacc
    import concourse.tile as tile
    from concourse import mybir

    dt = mybir.dt
    f32, bf16, i16 = dt.float32, dt.bfloat16, dt.int16
    AF = mybir.ActivationFunctionType
    OP = mybir.AluOpType

    nc = bacc.Bacc("TRN2", target_bir_lowering=False, debug=False,
                   enable_asserts=True, num_devices=NCORES)

    # ---- I/O ----
    nT_in = {s: nc.dram_tensor(f"nT_{s}", [C, NPAD], bf16,
                               kind="ExternalInput").ap() for s in "LR"}
    wkvT = nc.dram_tensor("wkvT", [128, 4 * 1024], bf16,
                          kind="ExternalInput").ap()
    woT = nc.dram_tensor("woT", [128, 4 * 512], bf16,
                         kind="ExternalInput").ap()
    boT_in = nc.dram_tensor("boT", [1, 512], bf16, kind="ExternalInput").ap()
    iota_in = nc.dram_tensor("iota", [128, 128], bf16,
                             kind="ExternalInput").ap()
    ident_in = nc.dram_tensor("ident", [128, 128], bf16,
                              kind="ExternalInput").ap()
    sidxK_in = {s: nc.dram_tensor(f"sidxK_{s}", [128, W * SCOL], i16,
                                  kind="ExternalInput").ap() for s in "LR"}
    didx_in = {s: nc.dram_tensor(f"didx_{s}", [128, W * DCOL], i16,
                                 kind="ExternalInput").ap() for s in "LR"}
    drel_in = {s: nc.dram_tensor(f"drel_{s}", [128, W * BPW], f32,
                                 kind="ExternalInput").ap() for s in "LR"}
    hT_out = {s: nc.dram_tensor(f"hT_{s}", [4, 128, W * BLK], bf16,
                                kind="ExternalOutput").ap() for s in "LR"}

    # ---- internal DRAM ----
    tkv_sh = {s: nc.dram_tensor(f"tkv_sh_{s}", [NPAD, 2 * C], bf16).ap()
              for s in "LR"}
    shared = "Shared" if NCORES > 4 else "Local"
    tkv = {s: nc.dram_tensor(f"tkv_{s}", [NCORES * NPAD, 2 * C], bf16,
                             addr_space=shared).ap() for s in "LR"}

    with tile.TileContext(nc) as tc:
        with tc.tile_pool(name="const", bufs=1) as cpool:
            wkvT_sb = cpool.tile([128, 4 * 1024], bf16)
            nc.sync.dma_start(wkvT_sb[:], wkvT[:, :])
            woT_sb = cpool.tile([128, 4 * 512], bf16)
            nc.sync.dma_start(woT_sb[:], woT[:, :])
            boT_sb = cpool.tile([1, 512], bf16)
            nc.sync.dma_start(boT_sb[:], boT_in[:, :])
            iota_sb = cpool.tile([128, 128], bf16)
            nc.sync.dma_start(iota_sb[:], iota_in[:, :])
            ident_sb = cpool.tile([128, 128], bf16)
            nc.sync.dma_start(ident_sb[:], ident_in[:, :])
            ones_col_b = cpool.tile([128, 1], bf16)
            nc.vector.memset(ones_col_b[:], 1.0)
            ones_row_b = cpool.tile([1, 128], bf16)
            nc.vector.memset(ones_row_b[:], 1.0)
            eps_sb = cpool.tile([128, 128], bf16)
            nc.vector.memset(eps_sb[:], 1e-32)
            idx_sb = {}
            for s in "LR":
                sK = cpool.tile([128, W * SCOL], i16, tag=f"sidxK{s}")
                nc.sync.dma_start(sK[:], sidxK_in[s][:, :])
                dK = cpool.tile([128, W * DCOL], i16, tag=f"didx{s}")
                nc.sync.dma_start(dK[:], didx_in[s][:, :])
                dr = cpool.tile([128, W * BPW], f32, tag=f"drel{s}")
                nc.sync.dma_start(dr[:], drel_in[s][:, :])
                idx_sb[s] = (sK, dK, dr)
            HR = 8
            hacc = {s: cpool.tile([128, 4, HR * BLK], bf16, tag=f"hacc{s}",
                                  name=f"hacc{s}")
                    for s in "LR"}

            # ---- phase A: projection GEMMs into table shards ----
            with (
                tc.tile_pool(name="feat", bufs=1) as fpool,
                tc.tile_pool(name="gemm_sb", bufs=4) as gsb,
                tc.tile_pool(name="psum_gemm", bufs=6, space="PSUM") as pg,
            ):
                for s in "LR":
                    feat = []
                    for cc in range(4):
                        t = fpool.tile([128, NPAD], bf16, tag=f"feat{s}{cc}")
                        nc.sync.dma_start(
                            t[:], nT_in[s][cc * 128:(cc + 1) * 128, :])
                        feat.append(t)
                    for ti in range(NTILES):
                        sb = gsb.tile([128, 1024], bf16)
                        for half in range(2):
                            ps = pg.tile([128, 512], f32)
                            for cc in range(4):
                                nc.tensor.matmul(
                                    ps[:],
                                    lhsT=feat[cc][:, ti * 128:(ti + 1) * 128],
                                    rhs=wkvT_sb[:, cc * 1024 + half * 512:
                                                cc * 1024 + half * 512 + 512],
                                    start=(cc == 0), stop=(cc == 3))
                            nc.scalar.copy(
                                sb[:, half * 512:(half + 1) * 512], ps[:])
                        nc.sync.dma_start(
                            tkv_sh[s][ti * 128:(ti + 1) * 128, :], sb[:])

                # ---- phase B: AllGather both tables ----
                if not SKIP_AG:
                    for s in "LR":
                        nc.gpsimd.collective_compute(
                            "AllGather", mybir.AluOpType.bypass,
                            replica_groups=[list(range(NCORES))],
                            ins=[tkv_sh[s]], outs=[tkv[s]])

            # ---- phase C: edge processing ----
            nidx_reg = nc.gpsimd.to_reg(WCAP)
            didx_reg = nc.gpsimd.to_reg(BLK)
            with (
                tc.tile_pool(name="gathK", bufs=3) as gkpool,
                tc.tile_pool(name="gathV", bufs=3) as gvpool,
                tc.tile_pool(name="gathD", bufs=3) as gdpool,
                tc.tile_pool(name="blk", bufs=8) as sp,
                tc.tile_pool(name="ohs", bufs=2 * BPW) as ohpool,
                tc.tile_pool(name="sem", bufs=4) as sep,
                tc.tile_pool(name="tail", bufs=4) as tp,
                # PSUM: 8 banks of [128, 512]xf32.  Small tiles are
                # packed as slices of a shared per-bank tile.
                tc.tile_pool(name="psS", bufs=2, space="PSUM") as psS,
                tc.tile_pool(name="psW", bufs=2, space="PSUM") as psW,
                tc.tile_pool(name="pmsg", bufs=2, space="PSUM") as pmsg,
                tc.tile_pool(name="ph", bufs=2, space="PSUM") as ph,
            ):
                sides = () if SKIP_C else (("L", "R"), ("R", "L"))
                FLUSH = 2
                flushed = 0
                for w in range(W):
                    if w - flushed >= FLUSH:
                        r0 = (flushed % HR) * BLK
                        for s in "LR":
                            for oc in range(4):
                                nc.sync.dma_start(
                                    hT_out[s][oc][:, flushed * BLK:w * BLK],
                                    hacc[s][:, oc,
                                            r0:r0 + (w - flushed) * BLK])
                        flushed = w
                    for s, o in sides:
                        sK_sb, dK_sb, drel_sb = idx_sb[s]
                        # gathers: dst K and src K channel-major, src V
                        # row-major
                        kdT = gdpool.tile([128, 4, BLK], bf16, tag=f"kd{s}")
                        ktg = gkpool.tile([128, 4, WCAP], bf16, tag=f"kt{s}")
                        if "no_tg" in DBG:
                            nc.vector.memset(kdT[:, 0, 0:1], 0.0)
                            nc.vector.memset(ktg[:, 0, 0:1], 0.0)
                        else:
                            nc.gpsimd.dma_gather(
                                kdT[:], tkv[s][:, 0:C],
                                dK_sb[:, w * DCOL:(w + 1) * DCOL],
                                BLK, didx_reg, C, elem_step=2 * C,
                                transpose=True)
                            nc.gpsimd.dma_gather(
                                ktg[:], tkv[o][:, 0:C],
                                sK_sb[:, w * SCOL:(w + 1) * SCOL],
                                WCAP, nidx_reg, C, elem_step=2 * C,
                                transpose=True)
                        vg = gvpool.tile([128, BPW, C], bf16, tag=f"v{s}")
                        nc.gpsimd.dma_gather(
                            vg[:], tkv[o][:, C:2 * C],
                            sK_sb[:, w * SCOL:(w + 1) * SCOL],
                            WCAP, nidx_reg, C, elem_step=2 * C)

                        # scores: S^T[e,n] per block; the per-edge score is
                        # the one-hot-masked row sum (one fused DVE op).
                        s_all = tp.tile([128, BPW], f32, tag="s_all")
                        for b in range(BPW):
                            St_ps = psS.tile([128, 128], f32,
                                             name="St_ps")[:]
                            for cc in range(4):
                                nc.tensor.matmul(
                                    St_ps,
                                    lhsT=ktg[:, cc, b * 128:(b + 1) * 128],
                                    rhs=kdT[:, cc, :],
                                    start=(cc == 0), stop=(cc == 3))
                            ohb = sp.tile([128, 128], bf16, tag="ohb")
                            nc.vector.tensor_scalar(
                                ohb[:], iota_sb[:],
                                drel_sb[:, w * BPW + b: w * BPW + b + 1],
                                None, op0=OP.is_equal)
                            se = sep.tile([128, 128], bf16, tag="se")
                            nc.vector.scalar_tensor_tensor(
                                se[:], St_ps, 1.0, ohb[:],
                                op0=OP.mult, op1=OP.mult,
                                accum_out=s_all[:, b:b + 1])
                        eh = tp.tile([128, BPW], f32, tag="eh")
                        nc.scalar.activation(eh[:], s_all[:], AF.Exp,
                                             scale=1.0 / TEMP)

                        # weighted one-hot scatter + z (each PSUM bank may
                        # hold only one pending accumulation group, so run
                        # groups to completion: cc-major over blocks)
                        msgT_ps = pmsg.tile([128, 4, 128], f32)
                        wtile = psW.tile([128, 512], f32)
                        z_ps = wtile[0:1, 0:128]
                        zbc_ps = wtile[:, 128:256]
                        ohs_list = []
                        for b in range(BPW):
                            ohs = ohpool.tile([128, 128], bf16, tag="ohs")
                            nc.vector.tensor_scalar(
                                ohs[:], iota_sb[:],
                                drel_sb[:, w * BPW + b: w * BPW + b + 1],
                                eh[:, b:b + 1], op0=OP.is_equal, op1=OP.mult)
                            ohs_list.append(ohs)
                        for cc in range(4):
                            for b in range(BPW):
                                nc.tensor.matmul(
                                    msgT_ps[:, cc, :],
                                    lhsT=vg[:, b, cc * 128:(cc + 1) * 128],
                                    rhs=ohs_list[b][:],
                                    start=(b == 0), stop=(b == BPW - 1))
                        for b in range(BPW):
                            nc.tensor.matmul(
                                z_ps, lhsT=ones_col_b[:], rhs=ohs_list[b][:],
                                start=(b == 0), stop=(b == BPW - 1))

                        # window tail: 1/z broadcast, divide, Wo, bias+lrelu
                        zm = tp.tile([1, 128], f32, tag="zm")
                        nc.vector.tensor_scalar_max(zm[:], z_ps, 1e-30)
                        zrb = tp.tile([1, 128], bf16, tag="zrb")
                        with nc.allow_low_precision(
                                reason="1/z broadcast in bf16, matches "
                                       "downstream bf16 divide"):
                            nc.vector.reciprocal(zrb[:], zm[:])
                        nc.tensor.matmul(zbc_ps, lhsT=ones_row_b[:],
                                         rhs=zrb[:], start=True, stop=True)
                        zbc = tp.tile([128, 128], bf16, tag="zbc")
                        nc.scalar.copy(zbc[:], zbc_ps)
                        msgT_sb = tp.tile([128, 4, 128], bf16, tag="msgT")
                        for cc in range(4):
                            nc.vector.tensor_tensor(
                                msgT_sb[:, cc, :], msgT_ps[:, cc, :],
                                zbc[:], op=OP.mult)
                        hT_ps = ph.tile([128, 4, 128], f32)
                        for oc in range(4):
                            for cc in range(4):
                                nc.tensor.matmul(
                                    hT_ps[:, oc, :],
                                    lhsT=woT_sb[:, cc * 512 + oc * 128:
                                                cc * 512 + oc * 128 + 128],
                                    rhs=msgT_sb[:, cc, :],
                                    start=(cc == 0), stop=False)
                            nc.tensor.matmul(
                                hT_ps[:, oc, :],
                                lhsT=boT_sb[:, oc * 128:(oc + 1) * 128],
                                rhs=ones_row_b[:], start=False, stop=True)
                        # leaky = 0.01*h + relu(0.99*h); Relu/Copy share
                        # the Exp act table (Lrelu does not -> reloads)
                        hr = tp.tile([128, 4, 128], bf16, tag="hr")
                        nc.scalar.activation(hr[:], hT_ps[:], AF.Relu,
                                             scale=1.0 - NEG)
                        h01 = tp.tile([128, 4, 128], bf16, tag="h01")
                        nc.scalar.activation(h01[:], hT_ps[:], AF.Copy,
                                             scale=NEG)
                        nc.vector.tensor_tensor(
                            hacc[s][:, :, (w % HR) * BLK:
                                    (w % HR) * BLK + BLK], hr[:],
                            h01[:], op=OP.add)
                if not SKIP_C:
                    r0 = (flushed % HR) * BLK
                    for s in "LR":
                        for oc in range(4):
                            nc.sync.dma_start(
                                hT_out[s][oc][:, flushed * BLK:],
                                hacc[s][:, oc,
                                        r0:r0 + (W - flushed) * BLK])

    nc.compile()
    return nc


def _host_inputs(inputs):
    import ml_dtypes
    bf16 = ml_dtypes.bfloat16

    nl = np.asarray(inputs["node_left"], np.float32)
    nr = np.asarray(inputs["node_right"], np.float32)
    Wk = np.asarray(inputs["Wk"], np.float32)
    Wv = np.asarray(inputs["Wv"], np.float32)
    Wo = np.asarray(inputs["Wo"], np.float32)
    bo = np.asarray(inputs["bo"], np.float32)
    sl = np.asarray(inputs["segmentation_index_left"], np.int64)
    sr = np.asarray(inputs["segmentation_index_right"], np.int64)

    coresL, wL = _prep_side(sl, sr)
    coresR, wR = _prep_side(sr, sl)
    W = max(wL, wR)
    arrL = _build_core_arrays(coresL, W)
    arrR = _build_core_arrays(coresR, W)

    Wkv = np.concatenate([Wk, Wv], 0)               # [1024, 512]
    WkvT = Wkv.T                                    # [512, 1024]
    wkvT_arr = np.zeros((128, 4 * 1024), np.float32)
    for cc in range(4):
        wkvT_arr[:, cc * 1024:(cc + 1) * 1024] = \
            WkvT[cc * 128:(cc + 1) * 128, :]
    woT_arr = np.zeros((128, 4 * 512), np.float32)
    for cc in range(4):
        for oc in range(4):
            woT_arr[:, cc * 512 + oc * 128: cc * 512 + (oc + 1) * 128] = \
                Wo[oc * 128:(oc + 1) * 128, cc * 128:(cc + 1) * 128].T
    boT_arr = bo.reshape(1, 512)                    # chunk-major [1, 512]
    iota_arr = np.broadcast_to(
        np.arange(128, dtype=np.float32)[None, :], (128, 128)).copy()
    ident_arr = np.eye(128, dtype=np.float32)

    def shardT(feat, c):
        sh = np.zeros((C, NPAD), np.float32)
        sh[:, :NPC] = feat[c * NPC:(c + 1) * NPC].T
        return np.ascontiguousarray(sh).astype(bf16)

    in_maps = []
    for c in range(NCORES):
        in_maps.append({
            "nT_L": shardT(nl, c),
            "nT_R": shardT(nr, c),
            "wkvT": wkvT_arr.astype(bf16),
            "woT": woT_arr.astype(bf16),
            "boT": boT_arr.astype(bf16),
            "iota": iota_arr.astype(bf16),
            "ident": ident_arr.astype(bf16),
            "sidxK_L": arrL[c]["sidxK"],
            "didx_L": arrL[c]["didx"],
            "drel_L": arrL[c]["drel"],
            "sidxK_R": arrR[c]["sidxK"],
            "didx_R": arrR[c]["didx"],
            "drel_R": arrR[c]["drel"],
        })
    return in_maps, arrL, arrR, W


def _assemble(results, arrs, key, W):
    out = np.zeros((N, C), np.float32)
    for c in range(NCORES):
        hT = np.asarray(results[c][key], np.float32).reshape(C, W * BLK)
        cn = arrs[c]["colnode"]
        m = cn >= 0
        out[cn[m]] = hT[:, m].T
    return out


_RUN_KWARGS = {}


def kernel(**inputs):
    from concourse.bass_utils import run_bass_kernel_spmd

    in_maps, arrL, arrR, W = _host_inputs(inputs)
    nc = _build_program(W)
    res = run_bass_kernel_spmd(nc, in_maps, core_ids=list(range(NCORES)),
                               **_RUN_KWARGS)
    out_l = _assemble(res.results, arrL, "hT_L", W)
    out_r = _assemble(res.results, arrR, "hT_R", W)
    kernel.last_results = res
    kernel.last_nc = nc
    kernel.last_W = W
    return (out_l, out_r)
